# revision 1
# baseline (speedup 1.0000x reference)
"""CTC loss (K.ctc_batch_cost, full lengths, blank=C-1) on 8 Trainium2 cores.

Pure data parallelism: 128 batch rows per core, batch on SBUF partitions.

The label gather is done by the DMA fabric, not compute engines: the host
passes y_pred transposed to [B, C, T] (bf16), and one indexed SWDGE
dma_gather per half fetches the 65 needed class-rows (512B each) per batch
straight into the batch-major, lattice-column-major SBUF layout
pl[b, l*256 + t]. No gpsimd ap_gather, no SBUF shuffle, no partition
permutation: gathered row i = l*128 + b lands on partition b, slot l.

Per rep (software-pipelined emission; rep N+1's front half hides under
rep N's DVE lattice):
  Pool: 13 dma_gather descriptor generations (640 rows each, sized under
        the SWDGE descriptor ring).
  Act : ln(mx) with accumulation -> smx; exp -> F; fused scale pass
        pl_s = pl*F + EPS*F (bf16).
  DVE : the CTC lattice: 129 rows as single tensor_tensor_scan recurrences
        alpha_t[s] = (c_t[s] + alpha_{t-1}[s]) * p_t[s]  (op0=add, op1=mult)
        from BOTH ends at once (fwd rows 0..64; bwd rows 128..65 as the
        mirrored recursion on H = p*G), one scalar_tensor_tensor per odd
        row for the skip term, plus the boundary splice.

Prescale: F = exp(C0 - mean_t ln sum_l p) per batch keeps the probability-
space alphas in fp32 range (F cancels exactly in the final loss; the sum
statistic is built by gpsimd TT-adds, which walrus supports on Pool).
loss = 256*ln(F) - ln(sum_t (alpha_t[64]+a65*alpha_t[63]) * H_{t+1}[65]).
"""
import numpy as np

B, T, C, L = 1024, 256, 100, 64
BLOC = 128               # batches per core
S1 = L + 1               # 65 lattice columns (64 labels + blank)
NIDX = BLOC * S1         # 8320 gathered rows per core
LA = 32                  # lattice columns in gather chunk A (rest in B)
EPS = 1e-7
C0 = 2.105               # calibration of the 17-col sum-stat proxy (nats/step)

_compiled = None


def make_idx(y_true_loc: np.ndarray) -> np.ndarray:
    """dma_gather index stream [128, 520] int16, wrapped+replicated.

    Gathered row i = l*128 + b fetches ypt row b*100 + class(b,l) so it
    lands on partition b, slot l. Stream element i sits at
    [16c + i%16, i//16] for every gpsimd core c.
    """
    i = np.arange(NIDX)
    l, b = i // BLOC, i % BLOC
    lab = np.minimum(l, L - 1)
    cls = np.where(l < L, y_true_loc[b, lab], C - 1)
    vals = (b * C + cls).astype(np.int16)
    idx = np.zeros((128, NIDX // 16), np.int16)
    for c in range(8):
        idx[16 * c + (i % 16), i // 16] = vals
    return idx


def core_in_map(ytc: np.ndarray, ypc: np.ndarray) -> dict:
    """Per-core inputs from y_true [128,64] int32, y_pred [128,256,100] f32."""
    import ml_dtypes
    ypt = np.ascontiguousarray(
        np.asarray(ypc, np.float32).transpose(0, 2, 1)
    ).reshape(BLOC * C, T).astype(ml_dtypes.bfloat16)
    return {"ypt": ypt, "yt": np.ascontiguousarray(ytc), "idxg": make_idx(ytc)}


def build(nc, repeats: int = 1):
    import concourse.mybir as mybir
    from concourse import tile

    f32 = mybir.dt.float32
    bf16 = mybir.dt.bfloat16
    Alu = mybir.AluOpType
    Act = mybir.ActivationFunctionType
    X = mybir.AxisListType.X

    ypt = nc.dram_tensor("ypt", [BLOC * C, T], bf16, kind="ExternalInput")
    yt = nc.dram_tensor("yt", [BLOC, L], mybir.dt.int32, kind="ExternalInput")
    idxg = nc.dram_tensor("idxg", [128, NIDX // 16], mybir.dt.int16,
                          kind="ExternalInput")
    loss = nc.dram_tensor("loss", [BLOC, 1], f32, kind="ExternalOutput")

    with tile.TileContext(nc) as tc:
        with (
            tc.tile_pool(name="praw", bufs=2) as praw_pool,
            tc.tile_pool(name="misc", bufs=1) as misc,
        ):
            idx_sb = misc.tile([128, NIDX // 16], mybir.dt.int16)
            yt_sb = misc.tile([128, L], mybir.dt.int32)
            allow = misc.tile([128, L], f32)
            pl2 = [misc.tile([128, T * S1], bf16, name=f"pl{i}")
                   for i in range(2)]                 # scaled, x2 (rep parity)
            mx = misc.tile([128, T], f32)
            smx = misc.tile([128, 1], f32)
            lnF2 = [misc.tile([128, 1], f32, name=f"lnF{i}") for i in range(2)]
            Fb = misc.tile([128, 1], f32)
            biasEF = misc.tile([128, 1], f32)
            # lattice row tiles: col 0 is a pad (always 0), state t at col t+1
            R = [misc.tile([128, T + 1], f32, name=f"row{i}") for i in range(3)]
            bH = [misc.tile([128, T + 1], f32, name=f"bh{i}") for i in range(3)]
            ctile = [misc.tile([128, T], f32, name=f"ct{i}") for i in range(2)]
            cbtile = [misc.tile([128, T], f32, name=f"cb{i}") for i in range(2)]
            czero = misc.tile([128, T], f32)
            sptile = misc.tile([128, T - 1], f32)
            sztile = misc.tile([128, T - 1], f32)
            tot = misc.tile([128, 1], f32)
            logtot = misc.tile([128, 1], f32)
            loss_sb = misc.tile([128, 1], f32)

            nc.sync.dma_start(idx_sb[:], idxg.ap())
            nc.sync.dma_start(yt_sb[:], yt.ap())

            # allow[j] = 1 if j==0 else (y[j] != y[j-1])
            nc.vector.tensor_tensor(
                allow[:, 1:L], yt_sb[:, 1:L], yt_sb[:, 0:L - 1],
                op=Alu.not_equal)
            nc.vector.memset(allow[:, 0:1], 1.0)
            nc.vector.memset(czero[:], 0.0)
            for i in range(3):
                nc.vector.memset(R[i][:, 0:1], 0.0)
                nc.vector.memset(bH[i][:, 0:1], 0.0)

            ypt_rows = ypt.ap()          # [12800, 256] rows of 512B

            def front_half(_rep):
                plx = pl2[_rep % 2]
                praw = praw_pool.tile([128, T * S1], bf16,
                                      name=f"praw_{_rep}", tag="praw")
                # indexed row gather in 13 chunks of <=5 class-rows per
                # batch (640 descriptors, under the 1024-descriptor SWDGE
                # scratch ring); the running-sum chases each chunk
                for l0 in range(0, S1, 5):
                    l1 = min(l0 + 5, S1)
                    n = (l1 - l0) * BLOC
                    nc.gpsimd.dma_gather(
                        praw[:, l0 * T:l1 * T].rearrange(
                            "q (s e) -> q s e", e=T),
                        ypt_rows, idx_sb[:, l0 * 8:l1 * 8],
                        num_idxs=n, num_idxs_reg=n, elem_size=T)
                # prescale statistic: sum of 17 lattice columns (every 4th
                # label + blank) on DVE -- cheap, full-T, and the l-subset
                # bias/noise is absorbed by C0 (spread stays ~8 nats)
                nc.vector.tensor_tensor(
                    mx[:], praw[:, L * T:S1 * T], praw[:, 0:T], op=Alu.add)
                for l in range(4, L, 4):
                    nc.vector.tensor_tensor(
                        mx[:], mx[:], praw[:, l * T:(l + 1) * T], op=Alu.add)
                # F = exp(C0 - mean_t ln sum_l p)
                nc.scalar.activation(mx[:], mx[:], Act.Ln, accum_out=smx[:])
                lnF = lnF2[_rep % 2]
                nc.vector.tensor_scalar(
                    lnF[:], smx[:], -1.0 / T, C0, op0=Alu.mult, op1=Alu.add)
                nc.scalar.activation(Fb[:], lnF[:], Act.Exp)
                nc.vector.tensor_scalar(
                    biasEF[:], Fb[:], EPS, None, op0=Alu.mult)
                # fused prescale: pl = praw*F + EPS*F
                nc.scalar.activation(plx[:], praw[:], Act.Identity,
                                     scale=Fb[:], bias=biasEF[:])

            def back_half(_rep):
                plx = pl2[_rep % 2]

                def pcol(col):                   # [128,256] t ascending
                    return plx[:, col * T:(col + 1) * T]

                def pcol_rev(col):               # [128,256] t descending
                    if col == 0:
                        return plx[:, T - 1::-1]
                    return plx[:, col * T + T - 1:col * T - 1:-1]

                def emit_fwd(s):
                    col = s // 2 if s % 2 == 1 else L
                    if s == 0:
                        d = czero[:]
                    elif s % 2 == 0 or s == 1:
                        d = R[(s - 1) % 3][:, 0:T]
                    else:
                        ct = ctile[(s // 2) % 2]
                        nc.vector.scalar_tensor_tensor(
                            ct[:], R[(s - 2) % 3][:, 0:T],
                            allow[:, s // 2:s // 2 + 1],
                            R[(s - 1) % 3][:, 0:T],
                            op0=Alu.mult, op1=Alu.add)
                        d = ct[:]
                    nc.vector.tensor_tensor_scan(
                        R[s % 3][:, 1:T + 1], d, pcol(col),
                        1.0 if s < 2 else 0.0, op0=Alu.add, op1=Alu.mult)

                def emit_bwd(s):
                    col = s // 2 if s % 2 == 1 else L
                    if s == 128:
                        d = czero[:]
                    elif s % 2 == 0 or s == 127:
                        d = bH[(s + 1) % 3][:, 0:T]
                    else:
                        cb = cbtile[(s // 2) % 2]
                        nc.vector.scalar_tensor_tensor(
                            cb[:], bH[(s + 2) % 3][:, 0:T],
                            allow[:, (s + 2) // 2:(s + 2) // 2 + 1],
                            bH[(s + 1) % 3][:, 0:T],
                            op0=Alu.mult, op1=Alu.add)
                        d = cb[:]
                    nc.vector.tensor_tensor_scan(
                        bH[s % 3][:, 1:T + 1], d, pcol_rev(col),
                        1.0 if s >= 127 else 0.0, op0=Alu.add, op1=Alu.mult)

                for i in range(65):
                    emit_fwd(i)
                    if i < 64:
                        emit_bwd(128 - i)

                # splice: P*F^T = sum_t (a_t[64]+a65*a_t[63]) * H_{t+1}[65]
                nc.vector.scalar_tensor_tensor(
                    sptile[:], R[63 % 3][:, 1:T], allow[:, 32:33],
                    R[64 % 3][:, 1:T], op0=Alu.mult, op1=Alu.add)
                nc.vector.tensor_tensor(
                    sztile[:], sptile[:], bH[65 % 3][:, T - 1:0:-1],
                    op=Alu.mult)
                nc.vector.tensor_reduce(
                    tot[:], sztile[:], axis=X, op=Alu.add)
                nc.scalar.activation(logtot[:], tot[:], Act.Ln)
                nc.vector.scalar_tensor_tensor(
                    loss_sb[:], lnF2[_rep % 2][:], float(T), logtot[:],
                    op0=Alu.mult, op1=Alu.subtract)

            # software-pipelined emission: rep N+1's front half is queued
            # before rep N's lattice
            for _rep in range(repeats):
                front_half(_rep)
                if _rep >= 1:
                    back_half(_rep - 1)
            back_half(repeats - 1)
            nc.sync.dma_start(loss.ap(), loss_sb[:])
    nc.compile()
    return nc


def _get_compiled():
    global _compiled
    if _compiled is None:
        import concourse.bacc as bacc
        nc = bacc.Bacc("TRN2", target_bir_lowering=False, debug=False,
                       num_devices=1)
        _compiled = build(nc)
    return _compiled


def kernel(y_true: np.ndarray, y_pred: np.ndarray) -> np.ndarray:
    from concourse.bass_utils import run_bass_kernel_spmd

    nc = _get_compiled()
    y_true = np.asarray(y_true)
    y_pred = np.asarray(y_pred, dtype=np.float32)
    in_maps = []
    for c in range(8):
        sl = slice(c * BLOC, (c + 1) * BLOC)
        ytc = np.ascontiguousarray(y_true[sl]).astype(np.int32, copy=False)
        in_maps.append(core_in_map(ytc, y_pred[sl]))
    res = run_bass_kernel_spmd(nc, in_maps, core_ids=list(range(8)))
    return np.concatenate([res.results[c]["loss"] for c in range(8)], axis=0)

